# revision 1
# baseline (speedup 1.0000x reference)
"""Trainium2 Bass kernel for causal self-attention with RoPE and tanh scoring.

Reference computation (per batch b, head h):
    q,k = rope(split_heads(Q)), rope(split_heads(K)); v = split_heads(V)
    scores = q @ k^T / sqrt(hs);  att = tanh(where(causal, scores, -inf))
    (masked positions become tanh(-inf) = -1 and DO contribute -1 * v)
    out = att @ v

Sharding: 32 (b,h) pairs -> 4 per core across 8 cores.
Device layout: q,k are fed transposed as [hs, T] with the head dim
deinterleaved (even components first, odd second) so the RoPE pair-swap
becomes a rotate of partition halves. v stays [T, hs] natural.

On-device algorithm per (b,h), with S^T formulation (scoresT[tk, tq]):
  for q-chunk C (512 wide):
    for k-tile J in 0..4C+3 (lower triangle only):
      S^T chunk = kT_J^T-matmul (fp32r);  for diagonal-band tiles an extra
      accumulating matmul adds -1e4 to the strict upper triangle region,
      so tanh gives exactly -1 there.
      att^T = tanh(scale * S^T)  (ScalarE, PSUM->SBUF)
      out^T_C += v_J^T @ att^T   (accumulating matmul)
    out_sb = out^T_C + corr_C    (corr_C = -sum of v rows beyond the chunk,
                                  from tiny step-mask matmuls: the skipped
                                  fully-masked tiles contribute exactly -1*v)
Output is written as out^T [hs, T] per pair; the host transposes back.
"""

import sys

if "/opt/trn_rl_repo" not in sys.path:
    sys.path.insert(0, "/opt/trn_rl_repo")

import numpy as np

B, T, C_EMB = 2, 2048, 2048
NH, HS = 16, 128
NCORES = 8
PAIRS = (B * NH) // NCORES  # 4 (b,h) pairs per core
NQ = 512                    # q-chunk width (PSUM bank = 512 fp32)
NKT = 128                   # k-tile rows
JT = T // NKT               # 16 k-tiles
NCH = T // NQ               # 4 q-chunks
BIG = 1.0e4
SCALE = 1.0 / np.sqrt(HS)

def _host_consts():
    """Per-core constant tensors (identical on every core), f32 numpy."""
    i = np.arange(HS // 2, dtype=np.float64)
    freqs = 1.0 / 10000.0 ** (2.0 * i / HS)           # [64]
    t = np.arange(T, dtype=np.float64)
    ang = np.outer(freqs, t)                           # [64, T]
    cos = np.cos(ang)
    sin = np.sin(ang)
    rope_c = np.concatenate([cos, cos], axis=0).astype(np.float32)    # [128, T]
    rope_s = np.concatenate([-sin, sin], axis=0).astype(np.float32)   # [128, T]

    # mask lhsT[k, p] = -BIG if k <= p (upper triangular incl diag)
    mask_a = (-BIG * np.triu(np.ones((NKT, NKT)))).astype(np.float32)

    # mask rhs one-hot tiles, one per diagonal-band alignment r = J - 4C
    # S^T[p, f] needs -BIG iff (128J + p) > (512C + f)  <=>  p >= f - 128r + 1
    mask_b = np.zeros((NKT, NCH, NQ), np.float32)
    for r in range(NCH):
        for f in range(NQ):
            th = f - NKT * r + 1
            if th <= NKT - 1:
                mask_b[max(th, 0), r, f] = 1.0

    # step mask for corrections: SM[p, J, c] = -1 if (128J + p) >= 512(c+1)
    sm = np.zeros((NKT, JT, NCH), np.float32)
    for j in range(JT):
        for c in range(NCH):
            tk = j * NKT + np.arange(NKT)
            sm[:, j, c] = np.where(tk >= NQ * (c + 1), -1.0, 0.0)

    import ml_dtypes
    return {"rope_c": rope_c, "rope_s": rope_s,
            "mask_a": mask_a.astype(ml_dtypes.bfloat16),
            "mask_b": mask_b.astype(ml_dtypes.bfloat16), "sm": sm}


def _build_program(reps=1):
    import concourse.bacc as bacc
    import concourse.mybir as mybir
    import concourse.tile as tile

    F32R = mybir.dt.float32r
    F32 = mybir.dt.float32
    AFT = mybir.ActivationFunctionType

    nc = bacc.Bacc("TRN2", target_bir_lowering=False, debug=False)

    BF16 = mybir.dt.bfloat16
    qk_d = nc.dram_tensor("qkT", [PAIRS, 2, HS, T], F32R, kind="ExternalInput")
    v_d = nc.dram_tensor("v", [PAIRS, NKT, JT, HS], F32R, kind="ExternalInput")
    rc_d = nc.dram_tensor("rope_c", [HS, T], F32, kind="ExternalInput")
    rs_d = nc.dram_tensor("rope_s", [HS, T], F32, kind="ExternalInput")
    ma_d = nc.dram_tensor("mask_a", [NKT, NKT], BF16, kind="ExternalInput")
    mb_d = nc.dram_tensor("mask_b", [NKT, NCH, NQ], BF16, kind="ExternalInput")
    sm_d = nc.dram_tensor("sm", [NKT, JT, NCH], F32R, kind="ExternalInput")
    out_d = nc.dram_tensor("outT", [PAIRS, HS, T], F32, kind="ExternalOutput")

    with tile.TileContext(nc) as tc:
        with (
            tc.tile_pool(name="consts", bufs=1) as consts,
            tc.tile_pool(name="qc", bufs=9) as q_pool,
            tc.tile_pool(name="vp", bufs=8) as v_pool,
            tc.tile_pool(name="ropet", bufs=3) as t_pool,
            tc.tile_pool(name="att", bufs=5) as att_pool,
            tc.tile_pool(name="osb", bufs=3) as osb_pool,
            tc.tile_pool(name="corr", bufs=2) as corr_pool,
            tc.tile_pool(name="psS", bufs=3, space="PSUM") as psS,
            tc.tile_pool(name="psO", bufs=2, space="PSUM") as psO,
        ):
            rc = consts.tile([HS, T], F32)
            rs = consts.tile([HS, T], F32)
            ma = consts.tile([NKT, NKT], BF16)
            mb = consts.tile([NKT, NCH, NQ], BF16)
            sm = consts.tile([NKT, JT, NCH], F32R)
            nc.scalar.dma_start(out=ma, in_=ma_d.ap())
            nc.scalar.dma_start(out=mb, in_=mb_d.ap())
            nc.scalar.dma_start(out=sm, in_=sm_d.ap())
            for h0, h1 in ((0, T // 2), (T // 2, T)):
                nc.scalar.dma_start(out=rc[:, h0:h1], in_=rc_d.ap()[:, h0:h1])
                nc.scalar.dma_start(out=rs[:, h0:h1], in_=rs_d.ap()[:, h0:h1])

            import concourse.bass as bass

            def _bcast2(ap):
                """[HS, NQ] slice -> [HS, 2, NQ] with a 0-stride middle dim."""
                return bass.AP(tensor=ap.tensor, offset=ap.offset,
                               ap=[list(ap.ap[0]), [0, 2], list(ap.ap[1])])

            def _load_rope_chunk(g, ch):
                """Load a [HS, 2, NQ] q+k column chunk and apply RoPE."""
                sl = slice(ch * NQ, (ch + 1) * NQ)
                src = qk_d.ap()[g].rearrange("s p t -> p s t")   # [HS, 2, T]
                x = q_pool.tile([HS, 2, NQ], F32R, tag="qk")
                nc.sync.dma_start(out=x, in_=src[:, :, sl])
                # partition-rotated re-read of the same DRAM chunk:
                # rows 64..127 land on partitions 0..63 and vice versa
                xs = t_pool.tile([HS, 2, NQ], F32, tag="xs")
                srcf = src.bitcast(F32)
                nc.sync.dma_start(out=xs[0:64], in_=srcf[64:128, :, sl])
                nc.sync.dma_start(out=xs[64:128], in_=srcf[0:64, :, sl])
                t1 = t_pool.tile([HS, 2, NQ], F32, tag="t1")
                t2 = t_pool.tile([HS, 2, NQ], F32, tag="t2")
                nc.gpsimd.tensor_mul(t1, x.bitcast(F32), _bcast2(rc[:, sl]))
                nc.vector.tensor_mul(t2, xs, _bcast2(rs[:, sl]))
                nc.vector.tensor_add(x, t1, t2)
                return x

            def _load_pair(g):
                kch, qch, vq = [], [], []
                v_src = v_d.ap()[g]  # [NKT, JT, HS], contiguous per partition
                for ch in range(NCH):
                    x = _load_rope_chunk(g, ch)
                    qch.append(x[:, 0, :])
                    kch.append(x[:, 1, :])
                    vt = v_pool.tile([NKT, 4, HS], F32R, tag="v")
                    nc.sync.dma_start(out=vt, in_=v_src[:, 4 * ch:4 * ch + 4, :])
                    vq.append(vt)
                return kch, qch, vq

            def _one_pair(g, loaded, nxt):
                kch, qch, vq = loaded

                def v_of(j):
                    return vq[j // 4][:, j % 4, :]

                corr_sb = corr_pool.tile([HS, NCH], F32)

                def _emit_corr():
                    # corr[d, c] = -sum_{tk >= 512(c+1)} v[tk, d]
                    corr_ps = psO.tile([HS, NCH], mybir.dt.float32, tag="o")
                    for j in range(JT):
                        nc.tensor.matmul(corr_ps, v_of(j), sm[:, j, :],
                                         start=(j == 0), stop=(j == JT - 1))
                    nc.vector.tensor_copy(corr_sb, corr_ps)

                # ---- attention ----
                from collections import deque
                pending = deque()  # software pipeline: AV lags two groups

                def _emit_av(item):
                    o_ps, att, ja, jb, last, c = item
                    nc.tensor.matmul(o_ps, v_of(ja), att[:, 0, :],
                                     start=(ja == 0), stop=False)
                    nc.tensor.matmul(o_ps, v_of(jb), att[:, 1, :],
                                     start=False, stop=last)
                    if last:
                        # copy the finished chunk out right away so its
                        # PSUM bank frees as early as possible
                        o_sb = osb_pool.tile([HS, NQ], F32)
                        nc.vector.tensor_scalar_add(o_sb, o_ps,
                                                    corr_sb[:, c:c + 1])
                        nc.sync.dma_start(
                            out=out_d.ap()[g][:, c * NQ:(c + 1) * NQ],
                            in_=o_sb)

                for c in range(NCH):
                    n_j = 4 * c + 4  # k-tiles 0..4c+3
                    o_ps = psO.tile([HS, NQ], mybir.dt.float32, tag="o")
                    for jp in range(n_j // 2):
                        ja, jb = 2 * jp, 2 * jp + 1
                        # Band tiles (j >= 4c) need masking; the pair shares a
                        # 256-aligned compute window [off, NQ).
                        r_a, r_b = ja - 4 * c, jb - 4 * c
                        off = 256 if r_a >= 2 else 0
                        s = psS.tile([NKT, 2, NQ], mybir.dt.float32, tag="s")
                        for idx, j, r in ((0, ja, r_a), (1, jb, r_b)):
                            nc.tensor.matmul(
                                s[:, idx, off:],
                                kch[j // 4][:, (j % 4) * NKT:(j % 4 + 1) * NKT],
                                qch[c][:, off:],
                                start=True, stop=not (r >= 0))
                            if r >= 0:
                                # add -BIG above the causal boundary across
                                # [off, 128r+128) — everything left of the
                                # compute window is memset to -1 instead
                                moff = off
                                mn = 128 * r + 128 - moff
                                nc.tensor.matmul(
                                    s[:, idx, moff:moff + mn], ma,
                                    mb[:, r, moff:moff + mn],
                                    start=False, stop=True)
                        att = att_pool.tile([NKT, 2, NQ], F32R)
                        if off:
                            nc.gpsimd.memset(att[:, :, 0:off].bitcast(F32), -1.0)
                        nc.scalar.activation(att[:, :, off:], s[:, :, off:],
                                             AFT.Tanh, scale=float(SCALE))
                        pending.append((o_ps, att, ja, jb, jb == n_j - 1, c))
                        if len(pending) > 2:
                            _emit_av(pending.popleft())
                    if c == 0:
                        _emit_corr()
                        if nxt is not None:
                            # emit next pair's loads/RoPE so DMA/Pool/DVE
                            # fill them in during this pair's attention
                            nxt.append(_load_pair(g + 1))
                while pending:
                    _emit_av(pending.popleft())

            def _pairs_body():
                loaded = _load_pair(0)
                for g in range(PAIRS):
                    nxt = [] if g + 1 < PAIRS else None
                    _one_pair(g, loaded, nxt)
                    loaded = nxt[0] if nxt else None

            if reps == 1:
                _pairs_body()
            else:
                with tc.For_i(0, reps, 1,
                              hint_engines=(mybir.EngineType.PE,
                                            mybir.EngineType.Activation,
                                            mybir.EngineType.SP)):
                    _pairs_body()

    nc.compile()
    return nc


_PROGRAMS = {}


def _get_program(reps=1):
    if reps not in _PROGRAMS:
        _PROGRAMS[reps] = _build_program(reps)
    return _PROGRAMS[reps]


def _shard_inputs(Q, K, V):
    consts = _host_consts()
    d = np.arange(HS)
    perm = np.concatenate([d[0::2], d[1::2]])  # deinterleave head dim

    in_maps = []
    for core in range(NCORES):
        qkT = np.empty((PAIRS, 2, HS, T), np.float32)
        v = np.empty((PAIRS, NKT, JT, HS), np.float32)
        for slot in range(PAIRS):
            g = core * PAIRS + slot
            b, h = divmod(g, NH)
            cols = h * HS + np.arange(HS)
            qkT[slot, 0] = Q[b][:, cols[perm]].T
            qkT[slot, 1] = K[b][:, cols[perm]].T
            v[slot] = V[b][:, cols].reshape(JT, NKT, HS).transpose(1, 0, 2)
        in_maps.append({
            "qkT": np.ascontiguousarray(qkT),
            "v": np.ascontiguousarray(v),
            "rope_c": consts["rope_c"],
            "rope_s": consts["rope_s"],
            "mask_a": consts["mask_a"],
            "mask_b": consts["mask_b"],
            "sm": consts["sm"],
        })
    return in_maps


def _gather_outputs(per_core_outT):
    out = np.empty((B, T, C_EMB), np.float32)
    for core in range(NCORES):
        outT = per_core_outT[core]  # [PAIRS, HS, T]
        for slot in range(PAIRS):
            g = core * PAIRS + slot
            b, h = divmod(g, NH)
            out[b, :, h * HS:(h + 1) * HS] = outT[slot].T
    return out


def kernel(Q, K, V):
    from concourse.bass_utils import run_bass_kernel_spmd

    Q = np.asarray(Q, dtype=np.float32)
    K = np.asarray(K, dtype=np.float32)
    V = np.asarray(V, dtype=np.float32)

    nc = _get_program()
    in_maps = _shard_inputs(Q, K, V)
    res = run_bass_kernel_spmd(nc, in_maps, core_ids=list(range(NCORES)))
    return _gather_outputs([res.results[c]["outT"] for c in range(NCORES)])

